# revision 20
# baseline (speedup 1.0000x reference)
"""Multi-head attention Trainium2 kernel (8 NeuronCores, SPMD).

Problem: N=2, Lq=Lk=2048, D=1024, H=16 heads, causal + padding mask,
score scaling = sqrt(#valid keys per sentence).

Sharding: core c -> (n = c // 4, g = c % 4): batch n, head group g of 4
heads (256 feature columns). No cross-core communication; the host
assembles the per-core [2048, 256] outputs into [2, 2048, 1024].

Per-core pipeline (all on one NeuronCore):
  1. xq/xk arrive host-pre-transposed ([D, L] bf16): plain fast DMAs
     land d-major SBUF tiles directly (no xbar transposes). Front-end
     DMA issue is split across the two HWDGE rings (sync + scalar).
  2. A stream of dummy matmuls warms the PE HAM clock governor (cold
     1.2 GHz, warm 2.4 GHz; it re-throttles after any ~3.4us activity
     window with idle in it) while the first tiles stream in.
  3. Projections (bf16 matmuls, fp32 PSUM): QT/KT [256f, 2048s] bf16,
     V packed into bf16 Vtilde [kc][128, 4*65] with a ones column per
     head (softmax denominators for free).
  4. Scores ST[k, q] = KT-slice.T @ QT, exp on ACT (padding mask as
     the per-partition ACT bias; 1/sqrt(valid) folded into Wq on the
     host; causal diagonal = triangular zero of the exp output via
     gpsimd affine_select, off the st->exp critical path). The
     schedule is a cross-block software pipeline: each block's
     non-diagonal score chunks interleave into the K/V projection
     stream and the PREVIOUS block's PV phase, so the scalar engine
     enters every block with its exp backlog drained and the PE never
     idles long enough to re-throttle. Explicit dummy-matmul padding
     covers the two structurally ACT-bound pockets (C_0, A_3).
  5. PV: out[q, 0:64]+denom[q] = P'T-chunk.T @ Vtilde accumulated over
     k-chunks in PSUM (one bank per open chain - PSUM allows only one
     open accumulation group per 2KB bank; per chain the diagonal
     chunk is accumulated LAST so its exp hides behind bulk work);
     normalize by the ones column; store per head-pair.
"""

import sys

sys.path.insert(0, "/opt/trn_rl_repo")

import numpy as np
import ml_dtypes

import concourse.tile as tile
from concourse import bacc, mybir
from concourse.bass_utils import run_bass_kernel_spmd

F32 = mybir.dt.float32
BF16 = mybir.dt.bfloat16

L = 2048          # sequence length (q and k)
D = 1024          # model dim
FPC = 256         # features per core (4 heads x 64)
HPC = 4           # heads per core
SC = L // 128     # 16 seq chunks of 128
DC = D // 128     # 8 d chunks of 128
NB = L // 512     # 4 q-blocks of 512
NEG = -1.0e9
N_WARM = 6        # dummy matmuls to hold PE busy until real data lands


def build_program(reps=1):
    nc = bacc.Bacc("TRN2", target_bir_lowering=False, debug=False, num_devices=8)

    # host pre-arranged in SBUF tile order:
    #   x: [sb*128 (partition), dc, s] so one slab = one fully contiguous
    #      descriptor (8KB per partition line); w: [p, dc, f] ditto.
    xq_d = nc.dram_tensor("xq_bf", [NB * 128, DC, 512], BF16, kind="ExternalInput").ap()
    xk_d = nc.dram_tensor("xk_bf", [NB * 128, DC, 512], BF16, kind="ExternalInput").ap()
    wq_d = nc.dram_tensor("wq_t", [128, DC, FPC], BF16, kind="ExternalInput").ap()
    wk_d = nc.dram_tensor("wk_t", [128, DC, FPC], BF16, kind="ExternalInput").ap()
    wv_d = nc.dram_tensor("wv_t", [128, DC, FPC], BF16, kind="ExternalInput").ap()
    pb_d = nc.dram_tensor("pad_bias", [128, SC], F32, kind="ExternalInput").ap()
    out_d = nc.dram_tensor("out", [L, FPC], F32, kind="ExternalOutput").ap()

    with tile.TileContext(nc) as tc:
        with (
            tc.tile_pool(name="consts", bufs=1) as consts,
            tc.tile_pool(name="wpool", bufs=1) as wpool,
            tc.tile_pool(name="xt", bufs=1) as xt_pool,
            tc.tile_pool(name="qkv", bufs=1) as qkv,
            tc.tile_pool(name="pt", bufs=48) as pt_pool,
            tc.tile_pool(name="ostage", bufs=3) as out_pool,
            tc.tile_pool(name="small", bufs=4) as small_pool,
            tc.tile_pool(name="pstage", bufs=3) as pstage_pool,
            tc.tile_pool(name="ring", bufs=2, space="PSUM") as ring,
            tc.tile_pool(name="stps", bufs=3, space="PSUM") as st_pool,
        ):
          for _rep in range(reps):
            # ---- front end ------------------------------------------------
            # ACT warmup: trigger the exp table load at t~0 so the first
            # real exp doesn't pay the table-load latency. Issued on
            # scalar BEFORE its front-end DMA triggers.
            warm = small_pool.tile([128, 1], F32, tag="warm")
            warm2 = small_pool.tile([128, 1], F32, tag="warm")
            nc.vector.memset(warm, 0.0)
            nc.scalar.activation(warm2, warm, mybir.ActivationFunctionType.Exp)

            # bf16 dummy-matmul source: memset so the PE warmup has NO DMA
            # dependency and can start at t~0.5us (HAM un-throttles after
            # ~3.4us of sustained busy -> warm by ~4us, right when the
            # first real data lands).
            wu_src = consts.tile([128, 512], BF16)
            nc.vector.memset(wu_src, 0.0)

            # weights: [128 (d within chunk), dc, f]
            wq = wpool.tile([128, DC, FPC], BF16)
            wk = wpool.tile([128, DC, FPC], BF16)
            wv = wpool.tile([128, DC, FPC], BF16)

            # x transposed, slab-major: [128 (d in chunk), sb, dc, s] so a
            # slab load is fully contiguous on BOTH sides (8KB/partition)
            xqt = xt_pool.tile([128, NB, DC, 512], BF16)
            xkt = xt_pool.tile([128, NB, DC, 512], BF16)

            def load_x(eng, x_d, x_t, sb):
                # one contiguous descriptor per slab (host pre-arranged)
                eng.dma_start(out=x_t[:, sb], in_=x_d[128 * sb : 128 * (sb + 1)])

            # Staged DMA priority: the DMA fabric caps at ~420 GB/s TOTAL
            # and the SDMA engines round-robin across ALL queued transfers
            # (neither issue order nor ring choice gives priority - and
            # the Tile scheduler freely reorders unrelated dma_starts). So
            # each stage is gated by a tiny SBUF->SBUF "blocker" DMA that
            # READS a tile of the previous stage and WRITES a corner of a
            # tile the next stage will overwrite: the write-write conflict
            # pins the order, so stage N+1 stays off the fabric until
            # stage N has fully landed. The garbage corner is then
            # overwritten by the real transfer.
            def gate(src_tile, dst_tile):
                nc.sync.dma_start(
                    out=dst_tile, in_=src_tile
                )

            nc.sync.dma_start(out=wq, in_=wq_d)
            load_x(nc.sync, xq_d, xqt, 0)
            # stage 2 (wk, xk0, wv) waits for xq0
            gate(xqt[0:1, 0, 0, 0:16], wk[0:1, 0, 0:16])
            gate(xqt[0:1, 0, 0, 16:32], xkt[0:1, 0, 0, 0:16])
            gate(xqt[0:1, 0, 0, 32:48], wv[0:1, 0, 0:16])
            nc.sync.dma_start(out=wk, in_=wk_d)
            load_x(nc.sync, xk_d, xkt, 0)
            nc.sync.dma_start(out=wv, in_=wv_d)
            # stage 3 (xq1, xk1) waits for xk0
            gate(xkt[0:1, 0, 0, 32:48], xqt[0:1, 1, 0, 0:16])
            gate(xkt[0:1, 0, 0, 48:64], xkt[0:1, 1, 0, 0:16])
            load_x(nc.sync, xq_d, xqt, 1)
            load_x(nc.sync, xk_d, xkt, 1)
            # stage 4 (remaining slabs) waits for xq1
            gate(xqt[0:1, 1, 0, 16:32], xqt[0:1, 2, 0, 0:16])
            gate(xqt[0:1, 1, 0, 32:48], xkt[0:1, 2, 0, 0:16])
            load_x(nc.sync, xq_d, xqt, 2)
            load_x(nc.sync, xk_d, xkt, 2)
            load_x(nc.sync, xq_d, xqt, 3)
            load_x(nc.sync, xk_d, xkt, 3)
            # scalar ring: just the (tiny) padding bias, needed by exp ~19us
            pad_bias = consts.tile([128, SC], F32)
            nc.scalar.dma_start(out=pad_bias, in_=pb_d)

            # PE warmup: dummy matmuls with no input dependency so HAM
            # un-throttles (~3.4us of SUSTAINED busy) while the first DMAs
            # are still in flight. Dead PSUM writes, never read.
            for i in range(N_WARM):
                wu_ps = ring.tile([128, 512], F32, tag="bank", name="wu_ps")
                nc.tensor.matmul(
                    wu_ps, lhsT=wu_src[:, 0:128], rhs=wu_src,
                    start=True, stop=True,
                )

            # projection outputs
            qt = qkv.tile([128, 2, L], BF16)   # [f within chunk, fc, q]
            kt = qkv.tile([128, 2, L], BF16)   # [f within chunk, fc, k]
            vt = qkv.tile([128, SC, HPC * 65], BF16)  # [k in chunk, kc, h*65+f]
            nc.gpsimd.memset(vt, 1.0)  # ones columns (col 64 of each head)

            # ---- helpers -------------------------------------------------
            def qproj_fc(sb, fc):
                pq = ring.tile([128, 512], F32, tag="bank", name="pq")
                for dc in range(DC):
                    nc.tensor.matmul(
                        pq,
                        lhsT=wq[:, dc, 128 * fc : 128 * (fc + 1)],
                        rhs=xqt[:, sb, dc],
                        start=(dc == 0),
                        stop=(dc == DC - 1),
                    )
                nc.vector.tensor_copy(qt[:, fc, 512 * sb : 512 * (sb + 1)], pq)

            def kproj_fc(sb, fc):
                pk = ring.tile([128, 512], F32, tag="bank", name="pk")
                for dc in range(DC):
                    nc.tensor.matmul(
                        pk,
                        lhsT=wk[:, dc, 128 * fc : 128 * (fc + 1)],
                        rhs=xkt[:, sb, dc],
                        start=(dc == 0),
                        stop=(dc == DC - 1),
                    )
                nc.vector.tensor_copy(kt[:, fc, 512 * sb : 512 * (sb + 1)], pk)

            def vproj_kc(kc):
                # V = xkT.T @ wv   -> [k-seq, f]
                pv = ring.tile([128, 512], F32, tag="bank", name="pv")
                for dc in range(DC):
                    nc.tensor.matmul(
                        pv[:, 0:FPC],
                        lhsT=xkt[:, kc // 4, dc, 128 * (kc % 4) : 128 * (kc % 4 + 1)],
                        rhs=wv[:, dc, :],
                        start=(dc == 0),
                        stop=(dc == DC - 1),
                    )
                # scatter heads into vt (col 64 of each head stays 1.0)
                nc.vector.tensor_copy(
                    vt[:, kc, :].rearrange("p (h f) -> p h f", h=HPC)[:, :, 0:64],
                    pv[:, 0:FPC].rearrange("p (h f) -> p h f", h=HPC),
                )

            def st_exp(b, c, pts):
                qs = max(0, c - 4 * b) * 128  # skip fully-masked q cols
                width = 512 - qs
                for p in range(2):  # head pair = feature chunk
                    st = st_pool.tile([128, 2, 512], F32, tag="st")
                    for hh in range(2):
                        lo, hi = 64 * hh, 64 * (hh + 1)
                        nc.tensor.matmul(
                            st[:, hh, :],
                            lhsT=kt[lo:hi, p, 128 * c : 128 * (c + 1)],
                            rhs=qt[lo:hi, p, 512 * b : 512 * (b + 1)],
                            start=True,
                            stop=True,
                        )
                    pt = pt_pool.tile([128, 2, width], BF16, tag="pt")
                    nc.scalar.activation(
                        pt,
                        st[:, :, qs:],
                        mybir.ActivationFunctionType.Exp,
                        bias=pad_bias[:, c : c + 1],
                        scale=1.0,
                    )
                    if c >= 4 * b:
                        # causal: zero exp output where q < k inside the
                        # diagonal 128x128 tile (gpsimd, SBUF-only)
                        for hh in range(2):
                            sl = pt[:, hh, 0:128]
                            nc.gpsimd.affine_select(
                                out=sl,
                                in_=sl,
                                compare_op=mybir.AluOpType.is_ge,
                                fill=0.0,
                                base=0,
                                pattern=[[1, 128]],
                                channel_multiplier=-1,
                            )
                    pts[(c, p)] = (pt, qs)

            def pv_qchunk(b, j, pts):
                # Per head pair: bulk accumulation (c < qc) for both heads
                # first, the diagonal-chunk matmuls last, so the diagonal
                # exp latency hides behind the other head's bulk work.
                # PSUM is drained by a cheap copy into pstage (frees the
                # ring slot fast); normalization runs off SBUF afterwards.
                qc = 4 * b + j
                pstage = pstage_pool.tile([128, HPC, 65], F32, tag="ps")
                ostage = out_pool.tile([128, FPC], F32, tag="os")
                for hp in range(2):
                    pos = []
                    for h in (2 * hp, 2 * hp + 1):
                        p, hh = h // 2, h % 2
                        po = ring.tile([128, 65], F32, tag="bank", name="po")
                        for c in range(qc):
                            ptile, qs = pts[(c, p)]
                            lo = 128 * j - qs
                            nc.tensor.matmul(
                                po,
                                lhsT=ptile[:, hh, lo : lo + 128],
                                rhs=vt[:, c, 65 * h : 65 * (h + 1)],
                                start=(c == 0),
                                stop=False,
                            )
                        pos.append((h, po))
                    for h, po in pos:
                        p, hh = h // 2, h % 2
                        ptile, qs = pts[(qc, p)]
                        nc.tensor.matmul(
                            po,
                            lhsT=ptile[:, hh, 0:128],
                            rhs=vt[:, qc, 65 * h : 65 * (h + 1)],
                            start=(qc == 0),
                            stop=True,
                        )
                        nc.vector.tensor_copy(pstage[:, h, :], po)
                    # normalize + store this head pair now (halves the
                    # final-chunk tail: the DMA overlaps hp=1's chains).
                    # One broadcast tensor_mul covers both heads (the DVE
                    # is the PV-region serializer on short-chain blocks).
                    h0 = 2 * hp
                    rec = small_pool.tile([128, 2], F32, tag="rec")
                    nc.vector.reciprocal(rec, pstage[:, h0 : h0 + 2, 64])
                    nc.vector.tensor_mul(
                        ostage[:, 128 * hp : 128 * (hp + 1)].rearrange(
                            "p (h f) -> p h f", h=2
                        ),
                        pstage[:, h0 : h0 + 2, 0:64],
                        rec.unsqueeze(2).broadcast_to((128, 2, 64)),
                    )
                    nc.sync.dma_start(
                        out=out_d[
                            128 * qc : 128 * (qc + 1), 128 * hp : 128 * (hp + 1)
                        ],
                        in_=ostage[:, 128 * hp : 128 * (hp + 1)],
                    )

            # ---- schedule ------------------------------------------------
            # Explicit globally-balanced order. Principles:
            #  * PE (94us) > ACT (78us): PE must never idle. Score-exp
            #    chunks are spread between projection units so the exp
            #    backlog stays within the 3-tile st_pool (no PE stall) but
            #    ACT is never starved ahead of a PV diagonal dependency.
            #  * Each block's 4 diagonal score chunks issue as a group
            #    right after their kproj, so their exps complete during
            #    the following projection work - the PV chains (diag MM
            #    last) then never wait on ACT.
            #  * All projections/scores of block b+1 that fit are pulled
            #    into block b's ACT-bound pockets (no dummy pad matmuls).
            pts = [dict() for _ in range(NB)]

            def st(b, c):
                st_exp(b, c, pts[b])

            # NOTE: program order IS semantic order in Tile (a read placed
            # before a write sees the old data) - every st(b, c) must come
            # textually after BOTH halves of the q/k projections it reads.
            # pt-pool liveness (bufs=48): a new pt tile's buffer slot must
            # belong to a tile whose LAST PV reader already ran - verified
            # by index: tile #n reuses #(n-48); at most 2 next-block score
            # chunks may sit inside a PV region before that block's own
            # tiles are freed.
            # (PE queues are in-order: never group >3 st-pool tiles without
            # ring-based PE work between them, or the PE stalls on exp.)
            qproj_fc(0, 0); qproj_fc(0, 1)
            kproj_fc(0, 0); kproj_fc(0, 1)
            st(0, 0); qproj_fc(1, 0); st(0, 1)              # diag 0    #0-7
            qproj_fc(1, 1); st(0, 2); st(0, 3)
            st(1, 0); vproj_kc(0); st(1, 1); vproj_kc(1)    # 8-15
            st(1, 2); vproj_kc(2); st(1, 3); vproj_kc(3)
            qproj_fc(2, 0); qproj_fc(2, 1)
            pv_qchunk(0, 0, pts[0]); st(2, 0)               # 16-23
            pv_qchunk(0, 1, pts[0]); st(2, 1)
            pv_qchunk(0, 2, pts[0]); st(2, 2)
            pv_qchunk(0, 3, pts[0]); st(2, 3)
            kproj_fc(1, 0); kproj_fc(1, 1)
            st(1, 4); vproj_kc(4); st(1, 5); vproj_kc(5)    # diag 1    24-31
            st(1, 6); vproj_kc(6); st(1, 7); vproj_kc(7)
            st(2, 4); qproj_fc(3, 0); st(2, 5); qproj_fc(3, 1)  # 32-35
            pv_qchunk(1, 0, pts[1]); st(2, 6)               # 36-43
            pv_qchunk(1, 1, pts[1]); st(2, 7)
            pv_qchunk(1, 2, pts[1]); st(3, 0)
            pv_qchunk(1, 3, pts[1]); st(3, 1)
            kproj_fc(2, 0); kproj_fc(2, 1)
            st(2, 8); vproj_kc(8); st(2, 9); vproj_kc(9)    # diag 2    44-51
            st(2, 10); vproj_kc(10); st(2, 11); vproj_kc(11)
            st(3, 2); kproj_fc(3, 0); st(3, 3); kproj_fc(3, 1)  # 52-59
            st(3, 4); st(3, 5)
            pv_qchunk(2, 0, pts[2]); st(3, 6)               # 60-63
            pv_qchunk(2, 1, pts[2]); st(3, 7)
            pv_qchunk(2, 2, pts[2])
            pv_qchunk(2, 3, pts[2])
            st(3, 8); vproj_kc(12); st(3, 9); vproj_kc(13)  # 64-71
            st(3, 10); vproj_kc(14); st(3, 11); vproj_kc(15)
            st(3, 12); st(3, 13)                            # diag 3    72-79
            pv_qchunk(3, 0, pts[3]); st(3, 14)
            pv_qchunk(3, 1, pts[3]); st(3, 15)
            pv_qchunk(3, 2, pts[3])
            pv_qchunk(3, 3, pts[3])

    nc.compile()
    return nc


_NC_CACHE = None


def get_program():
    global _NC_CACHE
    if _NC_CACHE is None:
        _NC_CACHE = build_program()
    return _NC_CACHE


def make_in_maps(query, key, Wq, Wk, Wv, padding_mask):
    query = np.asarray(query, dtype=np.float32)
    key = np.asarray(key, dtype=np.float32)
    Wq = np.asarray(Wq, dtype=np.float32)
    Wk = np.asarray(Wk, dtype=np.float32)
    Wv = np.asarray(Wv, dtype=np.float32)
    padding_mask = np.asarray(padding_mask)
    bf = ml_dtypes.bfloat16

    def x_layout(xT):
        # [D, L] -> [sb*128 (p), dc, s]: slab sb is one contiguous block
        return np.ascontiguousarray(
            xT.reshape(DC, 128, NB, 512).transpose(2, 1, 0, 3).reshape(NB * 128, DC, 512)
        ).astype(bf)

    def w_layout(wT):
        # [D, FPC] -> [p, dc, f] (SBUF tile order, contiguous)
        return np.ascontiguousarray(wT.reshape(DC, 128, FPC).transpose(1, 0, 2)).astype(bf)

    # per-batch host pre-transpose (shared across the 4 head-group cores)
    xq_l = [x_layout(query[n].T) for n in range(2)]
    xk_l = [x_layout(key[n].T) for n in range(2)]

    in_maps = []
    for core in range(8):
        n, g = core // 4, core % 4
        valid = float((~padding_mask[n]).sum())
        inv_scale = 1.0 / np.sqrt(valid)
        sl = slice(g * FPC, (g + 1) * FPC)
        pad_bias = np.where(padding_mask[n], NEG, 0.0).astype(np.float32)
        in_maps.append(
            {
                "xq_bf": xq_l[n],
                "xk_bf": xk_l[n],
                "wq_t": w_layout((Wq[sl] * inv_scale).T),
                "wk_t": w_layout(Wk[sl].T),
                "wv_t": w_layout(Wv[sl].T),
                "pad_bias": np.ascontiguousarray(pad_bias.reshape(SC, 128).T),
            }
        )
    return in_maps


def kernel(query, key, Wq, Wk, Wv, mask, padding_mask, n_heads):
    nc = get_program()
    in_maps = make_in_maps(query, key, Wq, Wk, Wv, padding_mask)
    res = run_bass_kernel_spmd(nc, in_maps, core_ids=list(range(8)))
    out = np.empty((2, L, D), dtype=np.float32)
    for core in range(8):
        n, g = core // 4, core % 4
        out[n, :, g * FPC : (g + 1) * FPC] = res.results[core]["out"]
    return out



# revision 22
# speedup vs baseline: 1.0868x; 1.0868x over previous
"""Multi-head attention Trainium2 kernel (8 NeuronCores, SPMD).

Problem: N=2, Lq=Lk=2048, D=1024, H=16 heads, causal + padding mask,
score scaling = sqrt(#valid keys per sentence).

Sharding: core c -> (n = c // 4, g = c % 4): batch n, head group g of 4
heads (256 feature columns). No cross-core communication; the host
assembles the per-core [2048, 256] outputs into [2, 2048, 1024].

Per-core pipeline (all on one NeuronCore):
  1. xq/xk arrive host-pre-transposed ([D, L] bf16): plain fast DMAs
     land d-major SBUF tiles directly (no xbar transposes). Front-end
     DMA issue is split across the two HWDGE rings (sync + scalar).
  2. A stream of dummy matmuls warms the PE HAM clock governor (cold
     1.2 GHz, warm 2.4 GHz; it re-throttles after any ~3.4us activity
     window with idle in it) while the first tiles stream in.
  3. Projections (bf16 matmuls, fp32 PSUM): QT/KT [256f, 2048s] bf16,
     V packed into bf16 Vtilde [kc][128, 4*65] with a ones column per
     head (softmax denominators for free).
  4. Scores ST[k, q] = KT-slice.T @ QT, exp on ACT (padding mask as
     the per-partition ACT bias; 1/sqrt(valid) folded into Wq on the
     host; causal diagonal = triangular zero of the exp output via
     gpsimd affine_select, off the st->exp critical path). The
     schedule is a cross-block software pipeline: each block's
     non-diagonal score chunks interleave into the K/V projection
     stream and the PREVIOUS block's PV phase, so the scalar engine
     enters every block with its exp backlog drained and the PE never
     idles long enough to re-throttle. Explicit dummy-matmul padding
     covers the two structurally ACT-bound pockets (C_0, A_3).
  5. PV: out[q, 0:64]+denom[q] = P'T-chunk.T @ Vtilde accumulated over
     k-chunks in PSUM (one bank per open chain - PSUM allows only one
     open accumulation group per 2KB bank; per chain the diagonal
     chunk is accumulated LAST so its exp hides behind bulk work);
     normalize by the ones column; store per head-pair.
"""

import sys

sys.path.insert(0, "/opt/trn_rl_repo")

import numpy as np
import ml_dtypes

import concourse.tile as tile
from concourse import bacc, mybir
from concourse.bass_utils import run_bass_kernel_spmd
from concourse.tile_rust import add_dep_helper

F32 = mybir.dt.float32
BF16 = mybir.dt.bfloat16

L = 2048          # sequence length (q and k)
D = 1024          # model dim
FPC = 256         # features per core (4 heads x 64)
HPC = 4           # heads per core
SC = L // 128     # 16 seq chunks of 128
DC = D // 128     # 8 d chunks of 128
NB = L // 512     # 4 q-blocks of 512
NEG = -1.0e9
N_WARM = 6        # dummy matmuls to hold PE busy until real data lands


def build_program(reps=1):
    nc = bacc.Bacc("TRN2", target_bir_lowering=False, debug=False, num_devices=8)

    # host pre-arranged in SBUF tile order:
    #   x: [sb*128 (partition), dc, s] so one slab = one fully contiguous
    #      descriptor (8KB per partition line); w: [p, dc, f] ditto.
    xq_d = nc.dram_tensor("xq_bf", [NB * 128, DC, 512], BF16, kind="ExternalInput").ap()
    xk_d = nc.dram_tensor("xk_bf", [NB * 128, DC, 512], BF16, kind="ExternalInput").ap()
    wq_d = nc.dram_tensor("wq_t", [128, DC, FPC], BF16, kind="ExternalInput").ap()
    wk_d = nc.dram_tensor("wk_t", [128, DC, FPC], BF16, kind="ExternalInput").ap()
    wv_d = nc.dram_tensor("wv_t", [128, DC, FPC], BF16, kind="ExternalInput").ap()
    pb_d = nc.dram_tensor("pad_bias", [128, SC], F32, kind="ExternalInput").ap()
    out_d = nc.dram_tensor("out", [L, FPC], F32, kind="ExternalOutput").ap()

    with tile.TileContext(nc) as tc:
        with (
            tc.tile_pool(name="consts", bufs=1) as consts,
            tc.tile_pool(name="wpool", bufs=1) as wpool,
            tc.tile_pool(name="xt", bufs=1) as xt_pool,
            tc.tile_pool(name="qkv", bufs=1) as qkv,
            tc.tile_pool(name="pt", bufs=48) as pt_pool,
            tc.tile_pool(name="ostage", bufs=3) as out_pool,
            tc.tile_pool(name="small", bufs=4) as small_pool,
            tc.tile_pool(name="pstage", bufs=3) as pstage_pool,
            tc.tile_pool(name="ring", bufs=2, space="PSUM") as ring,
            tc.tile_pool(name="stps", bufs=3, space="PSUM") as st_pool,
        ):
          for _rep in range(reps):
            # ---- front end ------------------------------------------------
            # ACT warmup: trigger the exp table load at t~0 so the first
            # real exp doesn't pay the table-load latency. Issued on
            # scalar BEFORE its front-end DMA triggers.
            warm = small_pool.tile([128, 1], F32, tag="warm")
            warm2 = small_pool.tile([128, 1], F32, tag="warm")
            nc.vector.memset(warm, 0.0)
            nc.scalar.activation(warm2, warm, mybir.ActivationFunctionType.Exp)

            # bf16 dummy-matmul source: memset so the PE warmup has NO DMA
            # dependency and can start at t~0.5us (HAM un-throttles after
            # ~3.4us of sustained busy -> warm by ~4us, right when the
            # first real data lands).
            wu_src = consts.tile([128, 512], BF16)
            nc.vector.memset(wu_src, 0.0)

            # weights: [128 (d within chunk), dc, f]
            wq = wpool.tile([128, DC, FPC], BF16)
            wk = wpool.tile([128, DC, FPC], BF16)
            wv = wpool.tile([128, DC, FPC], BF16)

            # x transposed, slab-major: [128 (d in chunk), sb, dc, s] so a
            # slab load is fully contiguous on BOTH sides (8KB/partition)
            xqt = xt_pool.tile([128, NB, DC, 512], BF16)
            xkt = xt_pool.tile([128, NB, DC, 512], BF16)

            def load_x(eng, x_d, x_t, sb):
                # one contiguous descriptor per slab (host pre-arranged)
                return eng.dma_start(out=x_t[:, sb], in_=x_d[128 * sb : 128 * (sb + 1)])

            # Staged DMA priority: the DMA fabric caps at ~420 GB/s TOTAL
            # and the SDMA engines round-robin across ALL queued transfers
            # (neither issue order nor ring choice gives priority, and the
            # Tile scheduler freely reorders unrelated dma_starts). Stage
            # the transfers with EXPLICIT instruction deps so later loads
            # stay off the fabric until the bytes that gate the first
            # compute (wq + xq slab 0) have landed.
            nc.sync.dma_start(out=wq, in_=wq_d)
            i_xq0 = load_x(nc.sync, xq_d, xqt, 0)
            i_wk = nc.sync.dma_start(out=wk, in_=wk_d)
            i_xk0 = load_x(nc.sync, xk_d, xkt, 0)
            i_wv = nc.sync.dma_start(out=wv, in_=wv_d)
            for i2 in (i_wk, i_xk0, i_wv):
                add_dep_helper(i2.ins, i_xq0.ins, reason="dma stage 2 after xq0")
            i_xq1 = load_x(nc.sync, xq_d, xqt, 1)
            i_xk1 = load_x(nc.sync, xk_d, xkt, 1)
            for i3 in (i_xq1, i_xk1):
                add_dep_helper(i3.ins, i_xk0.ins, reason="dma stage 3 after xk0")
            for sb4 in range(2, NB):
                i4a = load_x(nc.sync, xq_d, xqt, sb4)
                i4b = load_x(nc.sync, xk_d, xkt, sb4)
                add_dep_helper(i4a.ins, i_xk1.ins, reason="dma stage 4 after xk1")
                add_dep_helper(i4b.ins, i_xk1.ins, reason="dma stage 4 after xk1")
            # scalar ring: just the (tiny) padding bias, needed by exp ~19us
            pad_bias = consts.tile([128, SC], F32)
            nc.scalar.dma_start(out=pad_bias, in_=pb_d)

            # PE warmup: dummy matmuls with no input dependency so HAM
            # un-throttles (~3.4us of SUSTAINED busy) while the first DMAs
            # are still in flight. Dead PSUM writes, never read.
            for i in range(N_WARM):
                wu_ps = ring.tile([128, 512], F32, tag="bank", name="wu_ps")
                nc.tensor.matmul(
                    wu_ps, lhsT=wu_src[:, 0:128], rhs=wu_src,
                    start=True, stop=True,
                )

            # projection outputs
            qt = qkv.tile([128, 2, L], BF16)   # [f within chunk, fc, q]
            kt = qkv.tile([128, 2, L], BF16)   # [f within chunk, fc, k]
            vt = qkv.tile([128, SC, HPC * 65], BF16)  # [k in chunk, kc, h*65+f]
            nc.gpsimd.memset(vt, 1.0)  # ones columns (col 64 of each head)

            # ---- helpers -------------------------------------------------
            def qproj_fc(sb, fc):
                pq = ring.tile([128, 512], F32, tag="bank", name="pq")
                for dc in range(DC):
                    nc.tensor.matmul(
                        pq,
                        lhsT=wq[:, dc, 128 * fc : 128 * (fc + 1)],
                        rhs=xqt[:, sb, dc],
                        start=(dc == 0),
                        stop=(dc == DC - 1),
                    )
                nc.vector.tensor_copy(qt[:, fc, 512 * sb : 512 * (sb + 1)], pq)

            def kproj_fc(sb, fc):
                pk = ring.tile([128, 512], F32, tag="bank", name="pk")
                for dc in range(DC):
                    nc.tensor.matmul(
                        pk,
                        lhsT=wk[:, dc, 128 * fc : 128 * (fc + 1)],
                        rhs=xkt[:, sb, dc],
                        start=(dc == 0),
                        stop=(dc == DC - 1),
                    )
                nc.vector.tensor_copy(kt[:, fc, 512 * sb : 512 * (sb + 1)], pk)

            def vproj_kc(kc):
                # V = xkT.T @ wv   -> [k-seq, f]
                pv = ring.tile([128, 512], F32, tag="bank", name="pv")
                for dc in range(DC):
                    nc.tensor.matmul(
                        pv[:, 0:FPC],
                        lhsT=xkt[:, kc // 4, dc, 128 * (kc % 4) : 128 * (kc % 4 + 1)],
                        rhs=wv[:, dc, :],
                        start=(dc == 0),
                        stop=(dc == DC - 1),
                    )
                # scatter heads into vt (col 64 of each head stays 1.0)
                nc.vector.tensor_copy(
                    vt[:, kc, :].rearrange("p (h f) -> p h f", h=HPC)[:, :, 0:64],
                    pv[:, 0:FPC].rearrange("p (h f) -> p h f", h=HPC),
                )

            def st_exp(b, c, pts):
                qs = max(0, c - 4 * b) * 128  # skip fully-masked q cols
                width = 512 - qs
                for p in range(2):  # head pair = feature chunk
                    st = st_pool.tile([128, 2, 512], F32, tag="st")
                    for hh in range(2):
                        lo, hi = 64 * hh, 64 * (hh + 1)
                        nc.tensor.matmul(
                            st[:, hh, :],
                            lhsT=kt[lo:hi, p, 128 * c : 128 * (c + 1)],
                            rhs=qt[lo:hi, p, 512 * b : 512 * (b + 1)],
                            start=True,
                            stop=True,
                        )
                    pt = pt_pool.tile([128, 2, width], BF16, tag="pt")
                    nc.scalar.activation(
                        pt,
                        st[:, :, qs:],
                        mybir.ActivationFunctionType.Exp,
                        bias=pad_bias[:, c : c + 1],
                        scale=1.0,
                    )
                    if c >= 4 * b:
                        # causal: zero exp output where q < k inside the
                        # diagonal 128x128 tile (gpsimd, SBUF-only)
                        for hh in range(2):
                            sl = pt[:, hh, 0:128]
                            nc.gpsimd.affine_select(
                                out=sl,
                                in_=sl,
                                compare_op=mybir.AluOpType.is_ge,
                                fill=0.0,
                                base=0,
                                pattern=[[1, 128]],
                                channel_multiplier=-1,
                            )
                    pts[(c, p)] = (pt, qs)

            def pv_qchunk(b, j, pts):
                # Per head pair: bulk accumulation (c < qc) for both heads
                # first, the diagonal-chunk matmuls last, so the diagonal
                # exp latency hides behind the other head's bulk work.
                # PSUM is drained by a cheap copy into pstage (frees the
                # ring slot fast); normalization runs off SBUF afterwards.
                qc = 4 * b + j
                pstage = pstage_pool.tile([128, HPC, 65], F32, tag="ps")
                ostage = out_pool.tile([128, FPC], F32, tag="os")
                for hp in range(2):
                    pos = []
                    for h in (2 * hp, 2 * hp + 1):
                        p, hh = h // 2, h % 2
                        po = ring.tile([128, 65], F32, tag="bank", name="po")
                        for c in range(qc):
                            ptile, qs = pts[(c, p)]
                            lo = 128 * j - qs
                            nc.tensor.matmul(
                                po,
                                lhsT=ptile[:, hh, lo : lo + 128],
                                rhs=vt[:, c, 65 * h : 65 * (h + 1)],
                                start=(c == 0),
                                stop=False,
                            )
                        pos.append((h, po))
                    for h, po in pos:
                        p, hh = h // 2, h % 2
                        ptile, qs = pts[(qc, p)]
                        nc.tensor.matmul(
                            po,
                            lhsT=ptile[:, hh, 0:128],
                            rhs=vt[:, qc, 65 * h : 65 * (h + 1)],
                            start=(qc == 0),
                            stop=True,
                        )
                        nc.vector.tensor_copy(pstage[:, h, :], po)
                    # normalize + store this head pair now (halves the
                    # final-chunk tail: the DMA overlaps hp=1's chains).
                    # One broadcast tensor_mul covers both heads (the DVE
                    # is the PV-region serializer on short-chain blocks).
                    h0 = 2 * hp
                    rec = small_pool.tile([128, 2], F32, tag="rec")
                    nc.vector.reciprocal(rec, pstage[:, h0 : h0 + 2, 64])
                    nc.vector.tensor_mul(
                        ostage[:, 128 * hp : 128 * (hp + 1)].rearrange(
                            "p (h f) -> p h f", h=2
                        ),
                        pstage[:, h0 : h0 + 2, 0:64],
                        rec.unsqueeze(2).broadcast_to((128, 2, 64)),
                    )
                    nc.sync.dma_start(
                        out=out_d[
                            128 * qc : 128 * (qc + 1), 128 * hp : 128 * (hp + 1)
                        ],
                        in_=ostage[:, 128 * hp : 128 * (hp + 1)],
                    )

            # ---- schedule ------------------------------------------------
            # Explicit globally-balanced order. Principles:
            #  * PE (94us) > ACT (78us): PE must never idle. Score-exp
            #    chunks are spread between projection units so the exp
            #    backlog stays within the 3-tile st_pool (no PE stall) but
            #    ACT is never starved ahead of a PV diagonal dependency.
            #  * Each block's 4 diagonal score chunks issue as a group
            #    right after their kproj, so their exps complete during
            #    the following projection work - the PV chains (diag MM
            #    last) then never wait on ACT.
            #  * All projections/scores of block b+1 that fit are pulled
            #    into block b's ACT-bound pockets (no dummy pad matmuls).
            pts = [dict() for _ in range(NB)]

            def st(b, c):
                st_exp(b, c, pts[b])

            # NOTE: program order IS semantic order in Tile (a read placed
            # before a write sees the old data) - every st(b, c) must come
            # textually after BOTH halves of the q/k projections it reads.
            # pt-pool liveness (bufs=48): a new pt tile's buffer slot must
            # belong to a tile whose LAST PV reader already ran - verified
            # by index: tile #n reuses #(n-48); at most 2 next-block score
            # chunks may sit inside a PV region before that block's own
            # tiles are freed.
            # (PE queues are in-order: never group >3 st-pool tiles without
            # ring-based PE work between them, or the PE stalls on exp.)
            qproj_fc(0, 0); qproj_fc(0, 1)
            kproj_fc(0, 0); kproj_fc(0, 1)
            st(0, 0); qproj_fc(1, 0); st(0, 1)              # diag 0    #0-7
            qproj_fc(1, 1); st(0, 2); st(0, 3)
            st(1, 0); vproj_kc(0); st(1, 1); vproj_kc(1)    # 8-15
            st(1, 2); vproj_kc(2); st(1, 3); vproj_kc(3)
            qproj_fc(2, 0); qproj_fc(2, 1)
            pv_qchunk(0, 0, pts[0]); st(2, 0)               # 16-23
            pv_qchunk(0, 1, pts[0]); st(2, 1)
            pv_qchunk(0, 2, pts[0]); st(2, 2)
            pv_qchunk(0, 3, pts[0]); st(2, 3)
            kproj_fc(1, 0); kproj_fc(1, 1)
            st(1, 4); vproj_kc(4); st(1, 5); vproj_kc(5)    # diag 1    24-31
            st(1, 6); vproj_kc(6); st(1, 7); vproj_kc(7)
            st(2, 4); qproj_fc(3, 0); st(2, 5); qproj_fc(3, 1)  # 32-35
            pv_qchunk(1, 0, pts[1]); st(2, 6)               # 36-43
            pv_qchunk(1, 1, pts[1]); st(2, 7)
            pv_qchunk(1, 2, pts[1]); st(3, 0)
            pv_qchunk(1, 3, pts[1]); st(3, 1)
            kproj_fc(2, 0); kproj_fc(2, 1)
            st(2, 8); vproj_kc(8); st(2, 9); vproj_kc(9)    # diag 2    44-51
            st(2, 10); vproj_kc(10); st(2, 11); vproj_kc(11)
            st(3, 2); kproj_fc(3, 0); st(3, 3); kproj_fc(3, 1)  # 52-59
            st(3, 4); st(3, 5)
            pv_qchunk(2, 0, pts[2]); st(3, 6)               # 60-63
            pv_qchunk(2, 1, pts[2]); st(3, 7)
            pv_qchunk(2, 2, pts[2])
            pv_qchunk(2, 3, pts[2])
            st(3, 8); vproj_kc(12); st(3, 9); vproj_kc(13)  # 64-71
            st(3, 10); vproj_kc(14); st(3, 11); vproj_kc(15)
            st(3, 12); st(3, 13)                            # diag 3    72-79
            pv_qchunk(3, 0, pts[3]); st(3, 14)
            pv_qchunk(3, 1, pts[3]); st(3, 15)
            pv_qchunk(3, 2, pts[3])
            pv_qchunk(3, 3, pts[3])

    nc.compile()
    return nc


_NC_CACHE = None


def get_program():
    global _NC_CACHE
    if _NC_CACHE is None:
        _NC_CACHE = build_program()
    return _NC_CACHE


def make_in_maps(query, key, Wq, Wk, Wv, padding_mask):
    query = np.asarray(query, dtype=np.float32)
    key = np.asarray(key, dtype=np.float32)
    Wq = np.asarray(Wq, dtype=np.float32)
    Wk = np.asarray(Wk, dtype=np.float32)
    Wv = np.asarray(Wv, dtype=np.float32)
    padding_mask = np.asarray(padding_mask)
    bf = ml_dtypes.bfloat16

    def x_layout(xT):
        # [D, L] -> [sb*128 (p), dc, s]: slab sb is one contiguous block
        return np.ascontiguousarray(
            xT.reshape(DC, 128, NB, 512).transpose(2, 1, 0, 3).reshape(NB * 128, DC, 512)
        ).astype(bf)

    def w_layout(wT):
        # [D, FPC] -> [p, dc, f] (SBUF tile order, contiguous)
        return np.ascontiguousarray(wT.reshape(DC, 128, FPC).transpose(1, 0, 2)).astype(bf)

    # per-batch host pre-transpose (shared across the 4 head-group cores)
    xq_l = [x_layout(query[n].T) for n in range(2)]
    xk_l = [x_layout(key[n].T) for n in range(2)]

    in_maps = []
    for core in range(8):
        n, g = core // 4, core % 4
        valid = float((~padding_mask[n]).sum())
        inv_scale = 1.0 / np.sqrt(valid)
        sl = slice(g * FPC, (g + 1) * FPC)
        pad_bias = np.where(padding_mask[n], NEG, 0.0).astype(np.float32)
        in_maps.append(
            {
                "xq_bf": xq_l[n],
                "xk_bf": xk_l[n],
                "wq_t": w_layout((Wq[sl] * inv_scale).T),
                "wk_t": w_layout(Wk[sl].T),
                "wv_t": w_layout(Wv[sl].T),
                "pad_bias": np.ascontiguousarray(pad_bias.reshape(SC, 128).T),
            }
        )
    return in_maps


def kernel(query, key, Wq, Wk, Wv, mask, padding_mask, n_heads):
    nc = get_program()
    in_maps = make_in_maps(query, key, Wq, Wk, Wv, padding_mask)
    res = run_bass_kernel_spmd(nc, in_maps, core_ids=list(range(8)))
    out = np.empty((2, L, D), dtype=np.float32)
    for core in range(8):
        n, g = core // 4, core % 4
        out[n, :, g * FPC : (g + 1) * FPC] = res.results[core]["out"]
    return out

